# revision 18
# baseline (speedup 1.0000x reference)
"""BarCachedCrossAttention Trainium2 kernel.

Sharding: 8 cores = 4 batches x 2 head-groups (8 heads / 512 channels each).
Per core, everything is computed in a transposed layout (partition = context
token for scores) so probs never need a transpose: U^T = V'^T @ P^T with a
ones-column in V' producing the softmax denominators for free.  The
instrument mask is applied by zeroing masked tokens' V' rows (including the
ones-column), so exp needs no per-token bias and batches into 1024-wide ACT
ops.  The K/V projection and the attention (scores -> exp -> U accumulation)
are fused per context slab so ACT exp overlaps projection matmuls.

Host-side folds (pure input prep, the heavy GEMMs all run on device):
  - instrument/bar embeddings are added into the context once on the host
    (the same gather the reference does), so no combo-table / one-hot
    correction matmuls are needed on device;
  - K-bias is dropped (exactly cancels in softmax over n);
  - Q-bias + current instrument embedding fold into a per-channel bias
    applied by the Q-projection's PSUM->SBUF activation;
  - V-bias passes through softmax unchanged (weights sum to 1) and folds
    with the output bias: bo_eff = bo + bv @ Wo.T.

fp16 operands everywhere (1 cyc/row matmuls, FWL weight loads, half DMA);
PSUM/U stay f32.  exp uses a constant -12 shift (cancels in U/Z) keeping the
probs near unity.  Probs and V' are bf16 (range-safe under exp).  Score
matmul head-pairs share one PSUM tile so both 64-row groups become ready
together and run concurrently on the PE (row-group tiling).  1/Z runs at
bf16 for range (Z spans e^-9..2e6).
"""

import sys

sys.path.insert(0, "/opt/trn_rl_repo")

import numpy as np

import concourse.bacc as bacc
import concourse.tile as tile
from concourse import mybir
from concourse.bass_utils import run_bass_kernel_spmd

B, T, N_CTX, H = 4, 512, 2048, 1024
NUM_HEADS, NUM_INSTRUMENTS, MAX_BARS = 16, 16, 8
HEAD_DIM = H // NUM_HEADS  # 64
HG = 2  # head groups (cores per batch)
CH = H // HG  # 512 channels per core
NH_G = NUM_HEADS // HG  # 8 heads per core
P = 128
F32 = mybir.dt.float32
BF16 = mybir.dt.bfloat16
DT = mybir.dt.float16
SHIFT = -12.0  # exp shift keeps bf16 probs in a comfortable range

KC = H // P  # 8 contraction chunks for projections
PT_CH = CH // P  # 4 partition tiles of channels
NT = N_CTX // P  # 16 context tiles of 128 tokens
TT = T // P  # 4 tiles of query tokens
SLABS = [512, 512, 512, 384, 128]  # context slab sizes (sum = N_CTX)

_compiled = None


def _build():
    nc = bacc.Bacc("TRN2", target_bir_lowering=False, debug=False, num_devices=8)

    qT_d = nc.dram_tensor("qT", [P, KC, T], DT, kind="ExternalInput")
    ctxT_d = nc.dram_tensor("ctxT", [P, KC, N_CTX], DT, kind="ExternalInput")
    wq_d = nc.dram_tensor("wqT", [P, KC, CH], DT, kind="ExternalInput")
    wk_d = nc.dram_tensor("wkT", [P, KC, CH], DT, kind="ExternalInput")
    wv_d = nc.dram_tensor("wvT", [P, KC, CH], DT, kind="ExternalInput")
    wo_d = nc.dram_tensor("woT", [P, PT_CH, H], DT, kind="ExternalInput")
    mb_d = nc.dram_tensor("mb", [P, NT], F32, kind="ExternalInput")
    bqe_d = nc.dram_tensor("bqe", [P, PT_CH], F32, kind="ExternalInput")
    out_d = nc.dram_tensor("out", [T, H], DT, kind="ExternalOutput")

    with tile.TileContext(nc) as tc:
        with (
            nc.allow_low_precision(reason="fp16 matmul operands; accum stays f32"),
            tc.tile_pool(name="persist", bufs=1) as pers,
        ):
            # ---- input DMA in need order, interleaved across the sync and
            # scalar queues in chunk order so the per-queue transfer
            # bandwidth is not the bottleneck for the Q-projection stream:
            # chunk pair j of qt and wq land roughly together, and each
            # queue only carries ~1MB of the critical first wave.
            qt = pers.tile([P, KC, T], DT, name="qt_in")
            wq = pers.tile([P, KC, CH], DT, name="wq")
            for j, k2 in enumerate(range(0, KC, 2)):
                qa, qb = (nc.sync, nc.scalar) if j % 2 == 0 else (nc.scalar, nc.sync)
                qa.dma_start(qt[:, k2 : k2 + 2, :], qT_d.ap()[:, k2 : k2 + 2, :])
                qb.dma_start(wq[:, k2 : k2 + 2, :], wq_d.ap()[:, k2 : k2 + 2, :])
            wk = pers.tile([P, KC, CH], DT, name="wk")
            wv = pers.tile([P, KC, CH], DT, name="wv")
            wo = pers.tile([P, PT_CH, H], DT, name="wo")
            for k2 in range(0, 4, 2):
                nc.sync.dma_start(wk[:, k2 : k2 + 2, :], wk_d.ap()[:, k2 : k2 + 2, :])
            bqe = pers.tile([P, PT_CH], F32, name="bqe")
            nc.scalar.dma_start(bqe[:], bqe_d.ap())
            mb = pers.tile([P, NT], F32, name="mb")
            nc.scalar.dma_start(mb[:], mb_d.ap())

            ones8 = pers.tile([P, NH_G], F32, name="ones8")
            nc.vector.memset(ones8[:], 1.0)
            shiftb = pers.tile([P, 1], F32, name="shiftb")
            nc.vector.memset(shiftb[:], SHIFT)
            # HAM warm-up fodder: defined values for dummy matmuls that keep
            # the PE busy during the input-DMA ramp so the 2.4GHz clock is
            # engaged before the real stream starts.
            dmy = pers.tile([P, 256], DT, name="dmy")
            nc.gpsimd.memset(dmy[:], 0.0)

            QT = [pers.tile([P, T], DT, name=f"qt{p}") for p in range(PT_CH)]
            OT = [pers.tile([P, T], DT, name=f"ot{p}") for p in range(PT_CH)]
            U = [
                pers.tile([HEAD_DIM + 1, 2, T], F32, name=f"u{hp}")
                for hp in range(NH_G // 2)
            ]
            ZS = [pers.tile([1, 2, 512], F32, name=f"zs{hp}") for hp in range(NH_G // 2)]
            RF = [pers.tile([1, 2, 512], F32, name=f"rf{hp}") for hp in range(NH_G // 2)]
            # 1/Z broadcast across the 64 head-dim partitions: done by the
            # (otherwise idle) gpsimd engine, replacing fp32 ones-matmul
            # broadcasts that cost ~2.2us of PE each on the critical tail.
            PSR = [
                pers.tile([HEAD_DIM, 2, T], F32, name=f"psr{hp}")
                for hp in range(NH_G // 2)
            ]

            # ---- Q projection (k-major: streams behind the chunked DMA) ----
            with tc.tile_pool(name="qps", bufs=1, space="PSUM") as qps:
                ps_q = [qps.tile([P, 512], F32, name=f"ps_q{p}") for p in range(PT_CH)]
                # dummy matmuls: ~3.5us of PE activity during the DMA ramp
                # flips the HAM clock gate to 8/8 before the first real
                # matmul; their output is overwritten by the k=0 start=True
                # matmul below.
                for _ in range(16):
                    nc.tensor.matmul(
                        ps_q[0][:64, :256], dmy[:, :64], dmy[:],
                        start=True, stop=True,
                    )
                for k in range(KC):
                    for p in range(PT_CH):
                        nc.tensor.matmul(
                            ps_q[p][:],
                            wq[:, k, p * P : (p + 1) * P],
                            qt[:, k, :],
                            start=(k == 0),
                            stop=(k == KC - 1),
                        )
                for p in range(PT_CH):
                    nc.scalar.activation(
                        QT[p][:], ps_q[p][:], mybir.ActivationFunctionType.Identity,
                        bias=bqe[:, p : p + 1], scale=1.0,
                    )

            # ---- fused K/V projection + attention, one context slab at a time ----
            NS = len(SLABS)
            offs = [sum(SLABS[:i]) for i in range(NS)]

            def emit_proj(si, kvsb, kvps, preloaded=None):
                """K^T and V' tiles for slab si; returns (kts, vts)."""
                n0, sl = offs[si], SLABS[si]
                s4n = sl // P
                if preloaded is None:
                    slab = slabp.tile([P, KC, 512], DT, name="slab")
                    nc.sync.dma_start(
                        slab[:, :, :sl], ctxT_d.ap()[:, :, n0 : n0 + sl]
                    )
                else:
                    slab = preloaded
                kts = []
                for p in range(PT_CH):
                    ps = kvps.tile([P, 512], F32, name="ps_kv")
                    for k in range(KC):
                        nc.tensor.matmul(
                            ps[:, :sl],
                            wk[:, k, p * P : (p + 1) * P],
                            slab[:, k, :sl],
                            start=(k == 0), stop=(k == KC - 1),
                        )
                    kt = kvsb.tile([P, 512], DT, name=f"kt{p}")
                    nc.vector.tensor_copy(kt[:, :sl], ps[:, :sl])
                    kts.append(kt)
                vts = []
                for s4 in range(s4n):
                    i = (n0 // P) + s4
                    psv = kvps.tile([P, 512], F32, name="ps_kv")
                    for k in range(KC):
                        nc.tensor.matmul(
                            psv[:],
                            slab[:, k, s4 * P : (s4 + 1) * P],
                            wv[:, k, :],
                            start=(k == 0), stop=(k == KC - 1),
                        )
                    vt = kvsb.tile([P, NH_G, HEAD_DIM + 1], BF16, name=f"v{s4}")
                    nc.vector.tensor_scalar_mul(
                        vt[:, :, :HEAD_DIM],
                        psv[:].rearrange("p (h d) -> p h d", d=HEAD_DIM),
                        mb[:, i : i + 1],
                    )
                    nc.vector.tensor_scalar_mul(
                        vt[:, :, HEAD_DIM], ones8[:], mb[:, i : i + 1]
                    )
                    vts.append(vt)
                return kts, vts

            with (
                tc.tile_pool(name="slab", bufs=2) as slabp,
                tc.tile_pool(name="kvsb", bufs=2) as kvsb,
                tc.tile_pool(name="ptp", bufs=4) as ptp,
                tc.tile_pool(name="kvps", bufs=2, space="PSUM") as kvps,
                tc.tile_pool(name="sps", bufs=2, space="PSUM") as sps,
                tc.tile_pool(name="ups", bufs=1, space="PSUM") as ups,
            ):
                def emit_norm(hp):
                    # hi=0 on the vector engine, hi=1 on gpsimd: the tail's
                    # normalization work is DVE-serialized otherwise.
                    nc.vector.tensor_tensor(
                        OT[hp][0:HEAD_DIM, :],
                        U[hp][:HEAD_DIM, 0, :],
                        PSR[hp][:, 0, :],
                        op=mybir.AluOpType.mult,
                    )
                    nc.gpsimd.tensor_mul(
                        OT[hp][HEAD_DIM : 2 * HEAD_DIM, :],
                        U[hp][:HEAD_DIM, 1, :],
                        PSR[hp][:, 1, :],
                    )

                # slab0 chunk pairs, the rest of wk, then wv and wo on the
                # gpsimd queue (idle at start).  Keeping these off the sync
                # queue lets the qt/wq transfers there run unobstructed.
                first_slab = slabp.tile([P, KC, 512], DT, name="slab")
                for k2 in range(0, KC, 2):
                    nc.gpsimd.dma_start(
                        first_slab[:, k2 : k2 + 2, :],
                        ctxT_d.ap()[:, k2 : k2 + 2, 0:512],
                    )
                for k2 in range(4, KC, 2):
                    nc.gpsimd.dma_start(
                        wk[:, k2 : k2 + 2, :], wk_d.ap()[:, k2 : k2 + 2, :]
                    )
                for k2 in range(0, KC, 2):
                    nc.gpsimd.dma_start(
                        wv[:, k2 : k2 + 2, :], wv_d.ap()[:, k2 : k2 + 2, :]
                    )
                nc.gpsimd.dma_start(wo[:], wo_d.ap())
                cur = emit_proj(0, kvsb, kvps, preloaded=first_slab)
                for ns in range(NS):
                    kts, vts = cur
                    s4n = SLABS[ns] // P
                    # attention: per (head pair, 128-token ctx chunk): the
                    # score pair shares one PSUM tile -> both row groups
                    # become ready together and run concurrently on the PE.
                    for hp in range(NH_G // 2):
                        psus = ups.tile([HEAD_DIM + 1, 2, 512], F32, name="ps_u")
                        for s4 in range(s4n):
                            pss = sps.tile([P, 2, 512], F32, name="ps_s")
                            pts = ptp.tile([P, 2, 512], BF16, name="pt")
                            for hi in range(2):
                                d0, d1 = hi * HEAD_DIM, (hi + 1) * HEAD_DIM
                                nc.tensor.matmul(
                                    pss[:, hi, :],
                                    kts[hp][d0:d1, s4 * P : (s4 + 1) * P],
                                    QT[hp][d0:d1, :],
                                    start=True, stop=True,
                                )
                            nc.scalar.activation(
                                pts[:], pss[:], mybir.ActivationFunctionType.Exp,
                                bias=shiftb[:], scale=0.125,
                            )
                            for hi in range(2):
                                nc.tensor.matmul(
                                    psus[:, hi, :],
                                    vts[s4][:, 2 * hp + hi, :],
                                    pts[:, hi, :],
                                    start=(s4 == 0), stop=(s4 == s4n - 1),
                                )
                        if ns == 0:
                            nc.vector.tensor_copy(U[hp][:], psus[:])
                        else:
                            nc.vector.tensor_add(U[hp][:], U[hp][:], psus[:])
                        if ns == NS - 1:
                            # normalization, software-pipelined one head pair
                            # behind the attention: the DVE reciprocal chain
                            # for hp runs under hp+1's attention, and the psr
                            # broadcast + OT multiply for hp-1 are emitted
                            # here so OT is written well before the O
                            # projection reads it.
                            nc.scalar.copy(
                                ZS[hp][:], U[hp][HEAD_DIM : HEAD_DIM + 1, :, :]
                            )
                            nc.vector.reciprocal_approx_fast(RF[hp][:], ZS[hp][:])
                            nc.gpsimd.partition_broadcast(PSR[hp][:], RF[hp][:])
                            if hp > 0:
                                emit_norm(hp - 1)
                    if ns + 1 < NS:
                        cur = emit_proj(ns + 1, kvsb, kvps)
                    else:
                        emit_norm(NH_G // 2 - 1)

            # ---- output projection ----
            # p-outer emission over all 8 PSUM banks: the p-chunk MMs for
            # head-pair p only need OT[p], so chunks for early head pairs
            # run while later pairs are still normalizing; only the final
            # p wave gates the output copies.
            with (
                tc.tile_pool(name="ob", bufs=4) as obp,
                tc.tile_pool(name="ops", bufs=1, space="PSUM") as ops,
            ):
                tiles = [(tt, o) for tt in range(TT) for o in range(2)]
                psos = [ops.tile([P, 512], F32, name=f"ps_o{i}") for i in range(8)]
                for p in range(PT_CH):
                    for i, (tt, o) in enumerate(tiles):
                        nc.tensor.matmul(
                            psos[i][:],
                            OT[p][:, tt * P : (tt + 1) * P],
                            wo[:, p, o * 512 : (o + 1) * 512],
                            start=(p == 0), stop=(p == PT_CH - 1),
                        )
                dmaq = [nc.sync, nc.gpsimd]
                for i, (tt, o) in enumerate(tiles):
                    ob = obp.tile([P, 512], DT, name="ob")
                    if o == 0:
                        nc.vector.tensor_copy(ob[:], psos[i][:])
                    else:
                        nc.scalar.copy(ob[:], psos[i][:])
                    dmaq[i % 2].dma_start(
                        out_d.ap()[tt * P : (tt + 1) * P, o * 512 : (o + 1) * 512],
                        ob[:],
                    )

    nc.compile()
    return nc


def _prep_inputs(query, context, instrument_ids, current_instrument_id, bar_offsets,
                 Wq, bq, Wk, bk, Wv, bv, Wo, bo, inst_emb, bar_emb):
    f32, f16 = np.float32, np.float16
    query = np.asarray(query, f32)
    context = np.asarray(context, f32)
    inst = np.asarray(instrument_ids).astype(np.int64)
    bars = np.clip(np.asarray(bar_offsets).astype(np.int64), 0, MAX_BARS - 1)
    cur = min(max(int(np.asarray(current_instrument_id)), 0), NUM_INSTRUMENTS - 1)
    Wq, Wk, Wv, Wo = (np.asarray(w, f32) for w in (Wq, Wk, Wv, Wo))
    bq, bv, bo = (np.asarray(b, f32) for b in (bq, bv, bo))
    inst_emb = np.asarray(inst_emb, f32)
    bar_emb = np.asarray(bar_emb, f32)

    def chunked(a):  # (H, X) -> (P, KC_a, X) with row k*P+p -> [p, k]
        kc = a.shape[0] // P
        return np.ascontiguousarray(a.reshape(kc, P, -1).transpose(1, 0, 2))

    # embeddings folded into the context on the host (input prep)
    ctx_e = context + inst_emb[inst] + bar_emb[bars]  # (B, N, H)
    bq_eff = bq + inst_emb[cur] @ Wq.T  # (H,)
    bo_eff = bo + bv @ Wo.T  # V-bias passes through softmax
    WqT = Wq.T.astype(f16)
    WkT = Wk.T.astype(f16)
    WvT = Wv.T.astype(f16)
    WoT = Wo.T.astype(f16)

    in_maps = []
    for b in range(B):
        qT = chunked(query[b].T.astype(f16))
        ctxT = chunked(ctx_e[b].T.astype(f16))
        mbv = np.where(inst[b] == cur, 0.0, 1.0).astype(f32)
        mbt = np.ascontiguousarray(mbv.reshape(NT, P).T)  # (128, NT)
        for g in range(HG):
            sl = slice(g * CH, (g + 1) * CH)
            in_maps.append({
                "qT": qT,
                "ctxT": ctxT,
                "wqT": chunked(WqT[:, sl]),
                "wkT": chunked(WkT[:, sl]),
                "wvT": chunked(WvT[:, sl]),
                "woT": chunked(WoT[sl, :]),
                "mb": mbt,
                "bqe": np.ascontiguousarray(bq_eff[sl].reshape(PT_CH, P).T),
            })
    return in_maps, bo_eff


def kernel(**inputs) -> np.ndarray:
    global _compiled
    if _compiled is None:
        _compiled = _build()
    in_maps, bo_eff = _prep_inputs(**inputs)
    res = run_bass_kernel_spmd(_compiled, in_maps, list(range(B * HG))).results
    out = np.empty((B, T, H), np.float32)
    for b in range(B):
        out[b] = (
            res[b * HG]["out"].astype(np.float32)
            + res[b * HG + 1]["out"].astype(np.float32)
            + bo_eff
        )
    return out



# revision 24
# speedup vs baseline: 1.1645x; 1.1645x over previous
"""BarCachedCrossAttention Trainium2 kernel.

Sharding: 8 cores = 4 batches x 2 head-groups (8 heads / 512 channels each).
Per core, everything is computed in a transposed layout (partition = context
token for scores) so probs never need a transpose: U^T = V'^T @ P^T with a
ones-column in V' producing the softmax denominators for free.  The
instrument mask is applied by zeroing masked tokens' V' rows (including the
ones-column), so exp needs no per-token bias and batches into 1024-wide ACT
ops.  The K/V projection and the attention (scores -> exp -> U accumulation)
are fused per context slab so ACT exp overlaps projection matmuls.

Host-side folds (pure input prep, the heavy GEMMs all run on device):
  - instrument/bar embeddings are added into the context once on the host
    (the same gather the reference does), so no combo-table / one-hot
    correction matmuls are needed on device;
  - K-bias is dropped (exactly cancels in softmax over n);
  - Q-bias + current instrument embedding fold into a per-channel bias
    applied by the Q-projection's PSUM->SBUF activation;
  - V-bias passes through softmax unchanged (weights sum to 1) and folds
    with the output bias: bo_eff = bo + bv @ Wo.T.

fp16 operands everywhere (1 cyc/row matmuls, FWL weight loads, half DMA);
PSUM/U stay f32.  exp uses a constant -12 shift (cancels in U/Z) keeping the
probs near unity.  Probs and V' are bf16 (range-safe under exp).  Score
matmul head-pairs share one PSUM tile so both 64-row groups become ready
together and run concurrently on the PE (row-group tiling).  1/Z runs at
bf16 for range (Z spans e^-9..2e6).
"""

import sys

sys.path.insert(0, "/opt/trn_rl_repo")

import numpy as np

import concourse.bacc as bacc
import concourse.tile as tile
from concourse import mybir
from concourse.bass_utils import run_bass_kernel_spmd

B, T, N_CTX, H = 4, 512, 2048, 1024
NUM_HEADS, NUM_INSTRUMENTS, MAX_BARS = 16, 16, 8
HEAD_DIM = H // NUM_HEADS  # 64
HG = 2  # head groups (cores per batch)
CH = H // HG  # 512 channels per core
NH_G = NUM_HEADS // HG  # 8 heads per core
P = 128
F32 = mybir.dt.float32
BF16 = mybir.dt.bfloat16
DT = mybir.dt.float16
SHIFT = -12.0  # exp shift keeps bf16 probs in a comfortable range

KC = H // P  # 8 contraction chunks for projections
PT_CH = CH // P  # 4 partition tiles of channels
NT = N_CTX // P  # 16 context tiles of 128 tokens
TT = T // P  # 4 tiles of query tokens
SLABS = [512, 512, 512, 384, 128]  # context slab sizes (sum = N_CTX)

_compiled = None


def _build():
    nc = bacc.Bacc("TRN2", target_bir_lowering=False, debug=False, num_devices=8)

    qT_d = nc.dram_tensor("qT", [P, KC, T], DT, kind="ExternalInput")
    ctxT_d = nc.dram_tensor("ctxT", [P, KC, N_CTX], DT, kind="ExternalInput")
    wq_d = nc.dram_tensor("wqT", [P, KC, CH], DT, kind="ExternalInput")
    wk_d = nc.dram_tensor("wkT", [P, KC, CH], DT, kind="ExternalInput")
    wv_d = nc.dram_tensor("wvT", [P, KC, CH], DT, kind="ExternalInput")
    wo_d = nc.dram_tensor("woT", [P, PT_CH, H], DT, kind="ExternalInput")
    mb_d = nc.dram_tensor("mb", [P, NT], F32, kind="ExternalInput")
    bqe_d = nc.dram_tensor("bqe", [P, PT_CH], F32, kind="ExternalInput")
    out_d = nc.dram_tensor("out", [T, H], DT, kind="ExternalOutput")

    with tile.TileContext(nc) as tc:
        with (
            nc.allow_low_precision(reason="fp16 matmul operands; accum stays f32"),
            tc.tile_pool(name="persist", bufs=1) as pers,
        ):
            # ---- input DMA in need order, interleaved across the sync and
            # scalar queues in chunk order so the per-queue transfer
            # bandwidth is not the bottleneck for the Q-projection stream:
            # chunk pair j of qt and wq land roughly together, and each
            # queue only carries ~1MB of the critical first wave.
            qt = pers.tile([P, KC, T], DT, name="qt_in")
            wq = pers.tile([P, KC, CH], DT, name="wq")
            for j, k2 in enumerate(range(0, KC, 2)):
                qa, qb = (nc.sync, nc.scalar) if j % 2 == 0 else (nc.scalar, nc.sync)
                qa.dma_start(qt[:, k2 : k2 + 2, :], qT_d.ap()[:, k2 : k2 + 2, :])
                qb.dma_start(wq[:, k2 : k2 + 2, :], wq_d.ap()[:, k2 : k2 + 2, :])
            wk = pers.tile([P, KC, CH], DT, name="wk")
            wv = pers.tile([P, KC, CH], DT, name="wv")
            wo = pers.tile([P, PT_CH, H], DT, name="wo")
            for k2 in range(0, 4, 2):
                nc.sync.dma_start(wk[:, k2 : k2 + 2, :], wk_d.ap()[:, k2 : k2 + 2, :])
            bqe = pers.tile([P, PT_CH], F32, name="bqe")
            nc.scalar.dma_start(bqe[:], bqe_d.ap())
            for k2 in range(4, KC, 2):
                nc.scalar.dma_start(wk[:, k2 : k2 + 2, :], wk_d.ap()[:, k2 : k2 + 2, :])
            mb = pers.tile([P, NT], F32, name="mb")
            nc.scalar.dma_start(mb[:], mb_d.ap())

            ones8 = pers.tile([P, NH_G], F32, name="ones8")
            nc.vector.memset(ones8[:], 1.0)
            shiftb = pers.tile([P, 1], F32, name="shiftb")
            nc.vector.memset(shiftb[:], SHIFT)
            # HAM warm-up fodder: defined values for dummy matmuls that keep
            # the PE busy during the input-DMA ramp so the 2.4GHz clock is
            # engaged before the real stream starts.
            dmy = pers.tile([P, 256], DT, name="dmy")
            nc.gpsimd.memset(dmy[:], 0.0)

            QT = [pers.tile([P, T], DT, name=f"qt{p}") for p in range(PT_CH)]
            OT = [pers.tile([P, T], DT, name=f"ot{p}") for p in range(PT_CH)]
            U = [
                pers.tile([HEAD_DIM + 1, 2, T], F32, name=f"u{hp}")
                for hp in range(NH_G // 2)
            ]
            ZS = [pers.tile([1, 2, 512], F32, name=f"zs{hp}") for hp in range(NH_G // 2)]
            RF = [pers.tile([1, 2, 512], F32, name=f"rf{hp}") for hp in range(NH_G // 2)]
            # fp16 copy of 1/Z for the cheap PE broadcast matmul (gpsimd
            # custom ops turned out to have multi-us dispatch latency, and
            # the original fp32 ones-matmul broadcast ran 4-pass at ~2.2us
            # of PE each; fp16 is one pass at ~0.2us).
            RF16 = [pers.tile([1, 2, 512], DT, name=f"rf16_{hp}") for hp in range(NH_G // 2)]
            ones1 = pers.tile([1, HEAD_DIM], DT, name="ones1")
            nc.vector.memset(ones1[:], 1.0)

            # ---- Q projection (k-major: streams behind the chunked DMA) ----
            with tc.tile_pool(name="qps", bufs=1, space="PSUM") as qps:
                ps_q = [qps.tile([P, 512], F32, name=f"ps_q{p}") for p in range(PT_CH)]
                # dummy matmuls: ~3.5us of PE activity during the DMA ramp
                # flips the HAM clock gate to 8/8 before the first real
                # matmul; their output is overwritten by the k=0 start=True
                # matmul below.
                for _ in range(16):
                    nc.tensor.matmul(
                        ps_q[0][:64, :256], dmy[:, :64], dmy[:],
                        start=True, stop=True,
                    )
                for k in range(KC):
                    for p in range(PT_CH):
                        nc.tensor.matmul(
                            ps_q[p][:],
                            wq[:, k, p * P : (p + 1) * P],
                            qt[:, k, :],
                            start=(k == 0),
                            stop=(k == KC - 1),
                        )
                for p in range(PT_CH):
                    nc.scalar.activation(
                        QT[p][:], ps_q[p][:], mybir.ActivationFunctionType.Identity,
                        bias=bqe[:, p : p + 1], scale=1.0,
                    )

            # ---- fused K/V projection + attention, one context slab at a time ----
            NS = len(SLABS)
            offs = [sum(SLABS[:i]) for i in range(NS)]

            def emit_proj(si, kvsb, kvps, preloaded=None):
                """K^T and V' tiles for slab si; returns (kts, vts)."""
                n0, sl = offs[si], SLABS[si]
                s4n = sl // P
                if preloaded is None:
                    slab = slabp.tile([P, KC, 512], DT, name="slab")
                    nc.sync.dma_start(
                        slab[:, :, :sl], ctxT_d.ap()[:, :, n0 : n0 + sl]
                    )
                else:
                    slab = preloaded
                kts = []
                for p in range(PT_CH):
                    ps = kvps.tile([P, 512], F32, name="ps_kv")
                    for k in range(KC):
                        nc.tensor.matmul(
                            ps[:, :sl],
                            wk[:, k, p * P : (p + 1) * P],
                            slab[:, k, :sl],
                            start=(k == 0), stop=(k == KC - 1),
                        )
                    kt = kvsb.tile([P, 512], DT, name=f"kt{p}")
                    nc.vector.tensor_copy(kt[:, :sl], ps[:, :sl])
                    kts.append(kt)
                vts = []
                for s4 in range(s4n):
                    i = (n0 // P) + s4
                    psv = kvps.tile([P, 512], F32, name="ps_kv")
                    for k in range(KC):
                        nc.tensor.matmul(
                            psv[:],
                            slab[:, k, s4 * P : (s4 + 1) * P],
                            wv[:, k, :],
                            start=(k == 0), stop=(k == KC - 1),
                        )
                    vt = kvsb.tile([P, NH_G, HEAD_DIM + 1], BF16, name=f"v{s4}")
                    nc.vector.tensor_scalar_mul(
                        vt[:, :, :HEAD_DIM],
                        psv[:].rearrange("p (h d) -> p h d", d=HEAD_DIM),
                        mb[:, i : i + 1],
                    )
                    nc.vector.tensor_scalar_mul(
                        vt[:, :, HEAD_DIM], ones8[:], mb[:, i : i + 1]
                    )
                    vts.append(vt)
                return kts, vts

            with (
                tc.tile_pool(name="slab", bufs=2) as slabp,
                tc.tile_pool(name="kvsb", bufs=2) as kvsb,
                tc.tile_pool(name="ptp", bufs=4) as ptp,
                tc.tile_pool(name="kvps", bufs=2, space="PSUM") as kvps,
                tc.tile_pool(name="sps", bufs=2, space="PSUM") as sps,
                tc.tile_pool(name="ups", bufs=1, space="PSUM") as ups,
            ):
                def emit_norm(hp):
                    # 1/Z broadcast across 64 partitions: two col-tiled fp16
                    # K=1 matmuls fill one PSUM tile in a single ~213ns PE
                    # slot, then the DVE scales U into OT.
                    psr = kvps.tile([P, 512], F32, name="ps_kv")
                    for hi in range(2):
                        nc.tensor.matmul(
                            psr[hi * HEAD_DIM : (hi + 1) * HEAD_DIM, :],
                            ones1[:],
                            RF16[hp][:, hi, :],
                            start=True, stop=True,
                            tile_position=(0, hi * HEAD_DIM),
                        )
                    for hi in range(2):
                        nc.vector.tensor_tensor(
                            OT[hp][hi * HEAD_DIM : (hi + 1) * HEAD_DIM, :],
                            U[hp][:HEAD_DIM, hi, :],
                            psr[hi * HEAD_DIM : (hi + 1) * HEAD_DIM, :],
                            op=mybir.AluOpType.mult,
                        )

                # slab0 chunk pairs, then wv and wo on the gpsimd queue
                # (idle at start).  Keeping these off the sync queue lets
                # the qt/wq transfers there run unobstructed.
                first_slab = slabp.tile([P, KC, 512], DT, name="slab")
                for k2 in range(0, KC, 2):
                    nc.gpsimd.dma_start(
                        first_slab[:, k2 : k2 + 2, :],
                        ctxT_d.ap()[:, k2 : k2 + 2, 0:512],
                    )
                for k2 in range(0, KC, 2):
                    nc.gpsimd.dma_start(
                        wv[:, k2 : k2 + 2, :], wv_d.ap()[:, k2 : k2 + 2, :]
                    )
                nc.gpsimd.dma_start(wo[:], wo_d.ap())
                cur = emit_proj(0, kvsb, kvps, preloaded=first_slab)
                for ns in range(NS):
                    kts, vts = cur
                    s4n = SLABS[ns] // P
                    # attention: per (head pair, 128-token ctx chunk): the
                    # score pair shares one PSUM tile -> both row groups
                    # become ready together and run concurrently on the PE.
                    for hp in range(NH_G // 2):
                        psus = [
                            ups.tile([HEAD_DIM + 1, 512], F32, name=f"ps_u{hi}")
                            for hi in range(2)
                        ]
                        for s4 in range(s4n):
                            pss = sps.tile([P, 2, 512], F32, name="ps_s")
                            pts = ptp.tile([P, 2, 512], BF16, name="pt")
                            for hi in range(2):
                                d0, d1 = hi * HEAD_DIM, (hi + 1) * HEAD_DIM
                                nc.tensor.matmul(
                                    pss[:, hi, :],
                                    kts[hp][d0:d1, s4 * P : (s4 + 1) * P],
                                    QT[hp][d0:d1, :],
                                    start=True, stop=True,
                                )
                            nc.scalar.activation(
                                pts[:], pss[:], mybir.ActivationFunctionType.Exp,
                                bias=shiftb[:], scale=0.125,
                            )
                            for hi in range(2):
                                nc.tensor.matmul(
                                    psus[hi][:],
                                    vts[s4][:, 2 * hp + hi, :],
                                    pts[:, hi, :],
                                    start=(s4 == 0), stop=(s4 == s4n - 1),
                                )
                        for hi in range(2):
                            if ns == 0:
                                nc.vector.tensor_copy(
                                    U[hp][:, hi, :], psus[hi][:]
                                )
                            else:
                                nc.vector.tensor_add(
                                    U[hp][:, hi, :], U[hp][:, hi, :], psus[hi][:]
                                )
                        if ns == NS - 1:
                            # normalization, software-pipelined one head pair
                            # behind the attention: the DVE reciprocal chain
                            # for hp runs under hp+1's attention, and the psr
                            # broadcast + OT multiply for hp-1 are emitted
                            # here so OT is written well before the O
                            # projection reads it.
                            nc.scalar.copy(
                                ZS[hp][:], U[hp][HEAD_DIM : HEAD_DIM + 1, :, :]
                            )
                            nc.vector.reciprocal_approx_fast(RF[hp][:], ZS[hp][:])
                            nc.scalar.copy(RF16[hp][:], RF[hp][:])
                            if hp > 0:
                                emit_norm(hp - 1)
                    if ns + 1 < NS:
                        cur = emit_proj(ns + 1, kvsb, kvps)
                    else:
                        emit_norm(NH_G // 2 - 1)

            # ---- output projection ----
            # p-outer emission over all 8 PSUM banks: the p-chunk MMs for
            # head-pair p only need OT[p], so chunks for early head pairs
            # run while later pairs are still normalizing; only the final
            # p wave gates the output copies.
            with (
                tc.tile_pool(name="ob", bufs=4) as obp,
                tc.tile_pool(name="ops", bufs=1, space="PSUM") as ops,
            ):
                tiles = [(tt, o) for tt in range(TT) for o in range(2)]
                psos = [ops.tile([P, 512], F32, name=f"ps_o{i}") for i in range(8)]
                for p in range(PT_CH):
                    for i, (tt, o) in enumerate(tiles):
                        nc.tensor.matmul(
                            psos[i][:],
                            OT[p][:, tt * P : (tt + 1) * P],
                            wo[:, p, o * 512 : (o + 1) * 512],
                            start=(p == 0), stop=(p == PT_CH - 1),
                        )
                dmaq = [nc.sync, nc.gpsimd]
                for i, (tt, o) in enumerate(tiles):
                    ob = obp.tile([P, 512], DT, name="ob")
                    if o == 0:
                        nc.vector.tensor_copy(ob[:], psos[i][:])
                    else:
                        nc.scalar.copy(ob[:], psos[i][:])
                    dmaq[i % 2].dma_start(
                        out_d.ap()[tt * P : (tt + 1) * P, o * 512 : (o + 1) * 512],
                        ob[:],
                    )

    nc.compile()
    return nc


def _prep_inputs(query, context, instrument_ids, current_instrument_id, bar_offsets,
                 Wq, bq, Wk, bk, Wv, bv, Wo, bo, inst_emb, bar_emb):
    f32, f16 = np.float32, np.float16
    query = np.asarray(query, f32)
    context = np.asarray(context, f32)
    inst = np.asarray(instrument_ids).astype(np.int64)
    bars = np.clip(np.asarray(bar_offsets).astype(np.int64), 0, MAX_BARS - 1)
    cur = min(max(int(np.asarray(current_instrument_id)), 0), NUM_INSTRUMENTS - 1)
    Wq, Wk, Wv, Wo = (np.asarray(w, f32) for w in (Wq, Wk, Wv, Wo))
    bq, bv, bo = (np.asarray(b, f32) for b in (bq, bv, bo))
    inst_emb = np.asarray(inst_emb, f32)
    bar_emb = np.asarray(bar_emb, f32)

    def chunked(a):  # (H, X) -> (P, KC_a, X) with row k*P+p -> [p, k]
        kc = a.shape[0] // P
        return np.ascontiguousarray(a.reshape(kc, P, -1).transpose(1, 0, 2))

    # embeddings folded into the context on the host (input prep)
    ctx_e = context + inst_emb[inst] + bar_emb[bars]  # (B, N, H)
    bq_eff = bq + inst_emb[cur] @ Wq.T  # (H,)
    bo_eff = bo + bv @ Wo.T  # V-bias passes through softmax
    WqT = Wq.T.astype(f16)
    WkT = Wk.T.astype(f16)
    WvT = Wv.T.astype(f16)
    WoT = Wo.T.astype(f16)

    in_maps = []
    for b in range(B):
        qT = chunked(query[b].T.astype(f16))
        ctxT = chunked(ctx_e[b].T.astype(f16))
        mbv = np.where(inst[b] == cur, 0.0, 1.0).astype(f32)
        mbt = np.ascontiguousarray(mbv.reshape(NT, P).T)  # (128, NT)
        for g in range(HG):
            sl = slice(g * CH, (g + 1) * CH)
            in_maps.append({
                "qT": qT,
                "ctxT": ctxT,
                "wqT": chunked(WqT[:, sl]),
                "wkT": chunked(WkT[:, sl]),
                "wvT": chunked(WvT[:, sl]),
                "woT": chunked(WoT[sl, :]),
                "mb": mbt,
                "bqe": np.ascontiguousarray(bq_eff[sl].reshape(PT_CH, P).T),
            })
    return in_maps, bo_eff


def kernel(**inputs) -> np.ndarray:
    global _compiled
    if _compiled is None:
        _compiled = _build()
    in_maps, bo_eff = _prep_inputs(**inputs)
    res = run_bass_kernel_spmd(_compiled, in_maps, list(range(B * HG))).results
    out = np.empty((B, T, H), np.float32)
    for b in range(B):
        out[b] = (
            res[b * HG]["out"].astype(np.float32)
            + res[b * HG + 1]["out"].astype(np.float32)
            + bo_eff
        )
    return out



# revision 29
# speedup vs baseline: 1.2014x; 1.0317x over previous
"""BarCachedCrossAttention Trainium2 kernel.

Sharding: 8 cores = 4 batches x 2 head-groups (8 heads / 512 channels each).
Per core, everything is computed in a transposed layout (partition = context
token for scores) so probs never need a transpose: U^T = V'^T @ P^T with a
ones-column in V' producing the softmax denominators for free.  The
instrument mask is applied by zeroing masked tokens' V' rows (including the
ones-column), so exp needs no per-token bias and batches into 1024-wide ACT
ops.  The K/V projection and the attention (scores -> exp -> U accumulation)
are fused per context slab so ACT exp overlaps projection matmuls.

Host-side folds (pure input prep, the heavy GEMMs all run on device):
  - instrument/bar embeddings are added into the context once on the host
    (the same gather the reference does), so no combo-table / one-hot
    correction matmuls are needed on device;
  - K-bias is dropped (exactly cancels in softmax over n);
  - Q-bias + current instrument embedding fold into a per-channel bias
    applied by the Q-projection's PSUM->SBUF activation;
  - V-bias passes through softmax unchanged (weights sum to 1) and folds
    with the output bias: bo_eff = bo + bv @ Wo.T.

fp16 operands everywhere (1 cyc/row matmuls, FWL weight loads, half DMA);
PSUM/U stay f32.  exp uses a constant -12 shift (cancels in U/Z) keeping the
probs near unity.  Probs and V' are bf16 (range-safe under exp).  Score
matmul head-pairs share one PSUM tile so both 64-row groups become ready
together and run concurrently on the PE (row-group tiling).  1/Z runs at
bf16 for range (Z spans e^-9..2e6).
"""

import sys

sys.path.insert(0, "/opt/trn_rl_repo")

import numpy as np

import concourse.bacc as bacc
import concourse.tile as tile
from concourse import mybir
from concourse.bass_utils import run_bass_kernel_spmd

B, T, N_CTX, H = 4, 512, 2048, 1024
NUM_HEADS, NUM_INSTRUMENTS, MAX_BARS = 16, 16, 8
HEAD_DIM = H // NUM_HEADS  # 64
HG = 2  # head groups (cores per batch)
CH = H // HG  # 512 channels per core
NH_G = NUM_HEADS // HG  # 8 heads per core
P = 128
F32 = mybir.dt.float32
BF16 = mybir.dt.bfloat16
DT = mybir.dt.float16
SHIFT = -12.0  # exp shift keeps bf16 probs in a comfortable range

KC = H // P  # 8 contraction chunks for projections
PT_CH = CH // P  # 4 partition tiles of channels
NT = N_CTX // P  # 16 context tiles of 128 tokens
TT = T // P  # 4 tiles of query tokens
SLABS = [512, 512, 512, 384, 128]  # context slab sizes (sum = N_CTX)

_compiled = None


def _build():
    nc = bacc.Bacc("TRN2", target_bir_lowering=False, debug=False, num_devices=8)

    qT_d = nc.dram_tensor("qT", [P, KC, T], DT, kind="ExternalInput")
    ctxT_d = nc.dram_tensor("ctxT", [P, KC, N_CTX], DT, kind="ExternalInput")
    wq_d = nc.dram_tensor("wqT", [P, KC, CH], DT, kind="ExternalInput")
    wk_d = nc.dram_tensor("wkT", [P, KC, CH], DT, kind="ExternalInput")
    wv_d = nc.dram_tensor("wvT", [P, KC, CH], DT, kind="ExternalInput")
    wo_d = nc.dram_tensor("woT", [P, PT_CH, H], DT, kind="ExternalInput")
    mb_d = nc.dram_tensor("mb", [P, NT], F32, kind="ExternalInput")
    bqe_d = nc.dram_tensor("bqe", [P, PT_CH], F32, kind="ExternalInput")
    out_d = nc.dram_tensor("out", [T, H], DT, kind="ExternalOutput")

    with tile.TileContext(nc) as tc:
        with (
            nc.allow_low_precision(reason="fp16 matmul operands; accum stays f32"),
            tc.tile_pool(name="persist", bufs=1) as pers,
        ):
            # ---- input DMA in need order, interleaved across the sync and
            # scalar queues in chunk order so the per-queue transfer
            # bandwidth is not the bottleneck for the Q-projection stream:
            # chunk pair j of qt and wq land roughly together, and each
            # queue only carries ~1MB of the critical first wave.
            qt = pers.tile([P, KC, T], DT, name="qt_in")
            wq = pers.tile([P, KC, CH], DT, name="wq")
            for j, k2 in enumerate(range(0, KC, 2)):
                qa, qb = (nc.sync, nc.scalar) if j % 2 == 0 else (nc.scalar, nc.sync)
                qa.dma_start(qt[:, k2 : k2 + 2, :], qT_d.ap()[:, k2 : k2 + 2, :])
                qb.dma_start(wq[:, k2 : k2 + 2, :], wq_d.ap()[:, k2 : k2 + 2, :])
            wk = pers.tile([P, KC, CH], DT, name="wk")
            wv = pers.tile([P, KC, CH], DT, name="wv")
            wo = pers.tile([P, PT_CH, H], DT, name="wo")
            for k2 in range(0, 4, 2):
                nc.sync.dma_start(wk[:, k2 : k2 + 2, :], wk_d.ap()[:, k2 : k2 + 2, :])
            bqe = pers.tile([P, PT_CH], F32, name="bqe")
            nc.scalar.dma_start(bqe[:], bqe_d.ap())
            for k2 in range(4, KC, 2):
                nc.scalar.dma_start(wk[:, k2 : k2 + 2, :], wk_d.ap()[:, k2 : k2 + 2, :])
            mb = pers.tile([P, NT], F32, name="mb")
            nc.scalar.dma_start(mb[:], mb_d.ap())

            ones8 = pers.tile([P, NH_G], F32, name="ones8")
            nc.vector.memset(ones8[:], 1.0)
            shiftb = pers.tile([P, 1], F32, name="shiftb")
            nc.vector.memset(shiftb[:], SHIFT)
            # HAM warm-up fodder: defined values for dummy matmuls that keep
            # the PE busy during the input-DMA ramp so the 2.4GHz clock is
            # engaged before the real stream starts.
            dmy = pers.tile([P, 256], DT, name="dmy")
            nc.gpsimd.memset(dmy[:], 0.0)

            QT = [pers.tile([P, T], DT, name=f"qt{p}") for p in range(PT_CH)]
            OT = [pers.tile([P, T], DT, name=f"ot{p}") for p in range(PT_CH)]
            U = [
                pers.tile([HEAD_DIM + 1, 2, T], F32, name=f"u{hp}")
                for hp in range(NH_G // 2)
            ]
            ZS = [pers.tile([1, 2, 512], F32, name=f"zs{hp}") for hp in range(NH_G // 2)]
            RF = [pers.tile([1, 2, 512], F32, name=f"rf{hp}") for hp in range(NH_G // 2)]
            # fp16 copy of 1/Z for the cheap PE broadcast matmul (gpsimd
            # custom ops turned out to have multi-us dispatch latency, and
            # the original fp32 ones-matmul broadcast ran 4-pass at ~2.2us
            # of PE each; fp16 is one pass at ~0.2us).
            RF16 = [pers.tile([1, 2, 512], DT, name=f"rf16_{hp}") for hp in range(NH_G // 2)]
            ones1 = pers.tile([1, HEAD_DIM], DT, name="ones1")
            nc.vector.memset(ones1[:], 1.0)

            # ---- Q projection (k-major: streams behind the chunked DMA) ----
            with tc.tile_pool(name="qps", bufs=1, space="PSUM") as qps:
                ps_q = [qps.tile([P, 512], F32, name=f"ps_q{p}") for p in range(PT_CH)]
                # dummy matmuls: ~3.5us of PE activity during the DMA ramp
                # flips the HAM clock gate to 8/8 before the first real
                # matmul; their output is overwritten by the k=0 start=True
                # matmul below.
                for _ in range(16):
                    nc.tensor.matmul(
                        ps_q[0][:64, :256], dmy[:, :64], dmy[:],
                        start=True, stop=True,
                    )
                for k in range(KC):
                    for p in range(PT_CH):
                        nc.tensor.matmul(
                            ps_q[p][:],
                            wq[:, k, p * P : (p + 1) * P],
                            qt[:, k, :],
                            start=(k == 0),
                            stop=(k == KC - 1),
                        )
                for p in range(PT_CH):
                    nc.scalar.activation(
                        QT[p][:], ps_q[p][:], mybir.ActivationFunctionType.Identity,
                        bias=bqe[:, p : p + 1], scale=1.0,
                    )

            # ---- fused K/V projection + attention, one context slab at a time ----
            NS = len(SLABS)
            offs = [sum(SLABS[:i]) for i in range(NS)]

            def emit_proj(si, kvsb, kvps, preloaded=None):
                """K^T and V' tiles for slab si; returns (kts, vts)."""
                n0, sl = offs[si], SLABS[si]
                s4n = sl // P
                if preloaded is None:
                    slab = slabp.tile([P, KC, 512], DT, name="slab")
                    nc.sync.dma_start(
                        slab[:, :, :sl], ctxT_d.ap()[:, :, n0 : n0 + sl]
                    )
                else:
                    slab = preloaded
                kts = []
                for p in range(PT_CH):
                    ps = kvps.tile([P, 512], F32, name="ps_kv")
                    for k in range(KC):
                        nc.tensor.matmul(
                            ps[:, :sl],
                            wk[:, k, p * P : (p + 1) * P],
                            slab[:, k, :sl],
                            start=(k == 0), stop=(k == KC - 1),
                        )
                    kt = kvsb.tile([P, 512], DT, name=f"kt{p}")
                    nc.vector.tensor_copy(kt[:, :sl], ps[:, :sl])
                    kts.append(kt)
                vts = []
                for s4 in range(s4n):
                    i = (n0 // P) + s4
                    psv = kvps.tile([P, 512], F32, name="ps_kv")
                    for k in range(KC):
                        nc.tensor.matmul(
                            psv[:],
                            slab[:, k, s4 * P : (s4 + 1) * P],
                            wv[:, k, :],
                            start=(k == 0), stop=(k == KC - 1),
                        )
                    vt = kvsb.tile([P, NH_G, HEAD_DIM + 1], BF16, name=f"v{s4}")
                    nc.vector.tensor_scalar_mul(
                        vt[:, :, :HEAD_DIM],
                        psv[:].rearrange("p (h d) -> p h d", d=HEAD_DIM),
                        mb[:, i : i + 1],
                    )
                    nc.vector.tensor_scalar_mul(
                        vt[:, :, HEAD_DIM], ones8[:], mb[:, i : i + 1]
                    )
                    vts.append(vt)
                return kts, vts

            with (
                tc.tile_pool(name="slab", bufs=2) as slabp,
                tc.tile_pool(name="kvsb", bufs=2) as kvsb,
                tc.tile_pool(name="ptp", bufs=4) as ptp,
                tc.tile_pool(name="kvps", bufs=2, space="PSUM") as kvps,
                tc.tile_pool(name="sps", bufs=2, space="PSUM") as sps,
                tc.tile_pool(name="ups", bufs=1, space="PSUM") as ups,
            ):
                def emit_norm(hp):
                    # 1/Z broadcast across 64 partitions: two col-tiled fp16
                    # K=1 matmuls fill one PSUM tile in a single ~213ns PE
                    # slot, then the DVE scales U into OT.
                    psr = kvps.tile([P, 512], F32, name="ps_kv")
                    for hi in range(2):
                        nc.tensor.matmul(
                            psr[hi * HEAD_DIM : (hi + 1) * HEAD_DIM, :],
                            ones1[:],
                            RF16[hp][:, hi, :],
                            start=True, stop=True,
                            tile_position=(0, hi * HEAD_DIM),
                        )
                    for hi in range(2):
                        nc.vector.tensor_tensor(
                            OT[hp][hi * HEAD_DIM : (hi + 1) * HEAD_DIM, :],
                            U[hp][:HEAD_DIM, hi, :],
                            psr[hi * HEAD_DIM : (hi + 1) * HEAD_DIM, :],
                            op=mybir.AluOpType.mult,
                        )

                # slab0 chunk pairs + wo behind the qt/wk traffic on sync,
                # wv behind the wq/wk traffic on scalar: strict need order
                # per queue so nothing steals HBM bandwidth from the
                # critical early transfers (a third queue running big
                # transfers concurrently measurably starved them).
                first_slab = slabp.tile([P, KC, 512], DT, name="slab")
                for k2 in range(0, KC, 2):
                    nc.sync.dma_start(
                        first_slab[:, k2 : k2 + 2, :],
                        ctxT_d.ap()[:, k2 : k2 + 2, 0:512],
                    )
                for k2 in range(0, KC, 2):
                    nc.scalar.dma_start(
                        wv[:, k2 : k2 + 2, :], wv_d.ap()[:, k2 : k2 + 2, :]
                    )
                nc.sync.dma_start(wo[:], wo_d.ap())
                cur = emit_proj(0, kvsb, kvps, preloaded=first_slab)
                for ns in range(NS):
                    kts, vts = cur
                    s4n = SLABS[ns] // P
                    # attention: per (head pair, 128-token ctx chunk): the
                    # score pair shares one PSUM tile -> both row groups
                    # become ready together and run concurrently on the PE.
                    for hp in range(NH_G // 2):
                        psus = [
                            ups.tile([HEAD_DIM + 1, 512], F32, name=f"ps_u{hi}")
                            for hi in range(2)
                        ]
                        for s4 in range(s4n):
                            pss = sps.tile([P, 2, 512], F32, name="ps_s")
                            pts = ptp.tile([P, 2, 512], BF16, name="pt")
                            for hi in range(2):
                                d0, d1 = hi * HEAD_DIM, (hi + 1) * HEAD_DIM
                                nc.tensor.matmul(
                                    pss[:, hi, :],
                                    kts[hp][d0:d1, s4 * P : (s4 + 1) * P],
                                    QT[hp][d0:d1, :],
                                    start=True, stop=True,
                                )
                            nc.scalar.activation(
                                pts[:], pss[:], mybir.ActivationFunctionType.Exp,
                                bias=shiftb[:], scale=0.125,
                            )
                            for hi in range(2):
                                nc.tensor.matmul(
                                    psus[hi][:],
                                    vts[s4][:, 2 * hp + hi, :],
                                    pts[:, hi, :],
                                    start=(s4 == 0), stop=(s4 == s4n - 1),
                                )
                        for hi in range(2):
                            if ns == 0:
                                nc.vector.tensor_copy(
                                    U[hp][:, hi, :], psus[hi][:]
                                )
                            else:
                                nc.vector.tensor_add(
                                    U[hp][:, hi, :], U[hp][:, hi, :], psus[hi][:]
                                )
                        if ns == NS - 1:
                            # normalization, software-pipelined one head pair
                            # behind the attention: the DVE reciprocal chain
                            # for hp runs under hp+1's attention, and the psr
                            # broadcast + OT multiply for hp-1 are emitted
                            # here so OT is written well before the O
                            # projection reads it.  The Z row of U moves to
                            # partition 0 via a tiny sync-queue DMA (the
                            # partition-64 custom-DVE read NaN'd on HW, and
                            # an ACT staging copy costs 1.1us of chain).
                            nc.sync.dma_start(
                                ZS[hp][:], U[hp][HEAD_DIM : HEAD_DIM + 1, :, :]
                            )
                            nc.vector.reciprocal_approx_fast(RF[hp][:], ZS[hp][:])
                            nc.scalar.copy(RF16[hp][:], RF[hp][:])
                            # dummy matmuls keep the PE's activity monitor
                            # from re-throttling the clock during the norm
                            # phase (the O projection would then run at half
                            # clock).
                            dmp = kvps.tile([P, 512], F32, name="ps_kv")
                            for _ in range(3):
                                nc.tensor.matmul(
                                    dmp[:, :256], dmy[:, :P], dmy[:],
                                    start=True, stop=True,
                                )
                            if hp > 0:
                                emit_norm(hp - 1)
                    if ns + 1 < NS:
                        cur = emit_proj(ns + 1, kvsb, kvps)
                    else:
                        emit_norm(NH_G // 2 - 1)

            # ---- output projection ----
            # p-outer emission over all 8 PSUM banks: the p-chunk MMs for
            # head-pair p only need OT[p], so chunks for early head pairs
            # run while later pairs are still normalizing; only the final
            # p wave gates the output copies.
            with (
                tc.tile_pool(name="ob", bufs=4) as obp,
                tc.tile_pool(name="ops", bufs=1, space="PSUM") as ops,
            ):
                tiles = [(tt, o) for tt in range(TT) for o in range(2)]
                psos = [ops.tile([P, 512], F32, name=f"ps_o{i}") for i in range(8)]
                for p in range(PT_CH):
                    for i, (tt, o) in enumerate(tiles):
                        nc.tensor.matmul(
                            psos[i][:],
                            OT[p][:, tt * P : (tt + 1) * P],
                            wo[:, p, o * 512 : (o + 1) * 512],
                            start=(p == 0), stop=(p == PT_CH - 1),
                        )
                dmaq = [nc.sync, nc.gpsimd]
                for i, (tt, o) in enumerate(tiles):
                    ob = obp.tile([P, 512], DT, name="ob")
                    if o == 0:
                        nc.vector.tensor_copy(ob[:], psos[i][:])
                    else:
                        nc.scalar.copy(ob[:], psos[i][:])
                    dmaq[i % 2].dma_start(
                        out_d.ap()[tt * P : (tt + 1) * P, o * 512 : (o + 1) * 512],
                        ob[:],
                    )

    nc.compile()
    return nc


def _prep_inputs(query, context, instrument_ids, current_instrument_id, bar_offsets,
                 Wq, bq, Wk, bk, Wv, bv, Wo, bo, inst_emb, bar_emb):
    f32, f16 = np.float32, np.float16
    query = np.asarray(query, f32)
    context = np.asarray(context, f32)
    inst = np.asarray(instrument_ids).astype(np.int64)
    bars = np.clip(np.asarray(bar_offsets).astype(np.int64), 0, MAX_BARS - 1)
    cur = min(max(int(np.asarray(current_instrument_id)), 0), NUM_INSTRUMENTS - 1)
    Wq, Wk, Wv, Wo = (np.asarray(w, f32) for w in (Wq, Wk, Wv, Wo))
    bq, bv, bo = (np.asarray(b, f32) for b in (bq, bv, bo))
    inst_emb = np.asarray(inst_emb, f32)
    bar_emb = np.asarray(bar_emb, f32)

    def chunked(a):  # (H, X) -> (P, KC_a, X) with row k*P+p -> [p, k]
        kc = a.shape[0] // P
        return np.ascontiguousarray(a.reshape(kc, P, -1).transpose(1, 0, 2))

    # embeddings folded into the context on the host (input prep)
    ctx_e = context + inst_emb[inst] + bar_emb[bars]  # (B, N, H)
    bq_eff = bq + inst_emb[cur] @ Wq.T  # (H,)
    bo_eff = bo + bv @ Wo.T  # V-bias passes through softmax
    WqT = Wq.T.astype(f16)
    WkT = Wk.T.astype(f16)
    WvT = Wv.T.astype(f16)
    WoT = Wo.T.astype(f16)

    in_maps = []
    for b in range(B):
        qT = chunked(query[b].T.astype(f16))
        ctxT = chunked(ctx_e[b].T.astype(f16))
        mbv = np.where(inst[b] == cur, 0.0, 1.0).astype(f32)
        mbt = np.ascontiguousarray(mbv.reshape(NT, P).T)  # (128, NT)
        for g in range(HG):
            sl = slice(g * CH, (g + 1) * CH)
            in_maps.append({
                "qT": qT,
                "ctxT": ctxT,
                "wqT": chunked(WqT[:, sl]),
                "wkT": chunked(WkT[:, sl]),
                "wvT": chunked(WvT[:, sl]),
                "woT": chunked(WoT[sl, :]),
                "mb": mbt,
                "bqe": np.ascontiguousarray(bq_eff[sl].reshape(PT_CH, P).T),
            })
    return in_maps, bo_eff


def kernel(**inputs) -> np.ndarray:
    global _compiled
    if _compiled is None:
        _compiled = _build()
    in_maps, bo_eff = _prep_inputs(**inputs)
    res = run_bass_kernel_spmd(_compiled, in_maps, list(range(B * HG))).results
    out = np.empty((B, T, H), np.float32)
    for b in range(B):
        out[b] = (
            res[b * HG]["out"].astype(np.float32)
            + res[b * HG + 1]["out"].astype(np.float32)
            + bo_eff
        )
    return out

